# revision 16
# baseline (speedup 1.0000x reference)
"""Trainium2 Bass kernel for nn_ExperimentalLayer9 (dense transformer layer).

Layer: x + gelu(attn(x)) @ Wf with
  Q = split_heads(x), K = split_heads(x@Wk+bk), V = split_heads(x@Wv+bv)
  causal softmax (no 1/sqrt(d) scale), exact-erf gelu, residual add.

Sharding over 8 NeuronCores: 2 batch groups x 4-way head/tensor parallel.
Core c handles batch b=c//4 and heads [4r, 4r+4) with r=c%4.

v5 pipeline (vs the v4 349us run):
  * Fully software-pipelined pair phases: the leading score strips of pair
    i+1 are emitted inside pair i's AV slots, with the exp buffer double-
    buffered (the second buffer reuses the xT staging slot, dead after the
    K projection + fp8 casts).  The per-pair "leading phase" is then pure
    PE filler (V tiles + previous group's FF sub-units + gelus on ACT),
    so the PE no longer stalls on ACT exp pacing -- v4's HAM traces showed
    131us at K=4/8 from exactly those bubbles.
  * All bulk loads moved to the Sync queue (wk first): multi-us DMA
    descriptor-generation slices were blocking the Scalar queue where the
    exps run; scalar keeps only 3 tiny const loads.
  * The last chunk's FF sub-units + RS trigger are folded into its AV
    phase half-by-half, shortening the tail.
  * 4 full-group 1MB ReduceScatters, drains software-pipelined one group
    behind (v4); residual added post-collective from a compact [512,D]
    slice; PE transpose-mode o^T; DVE-fused K/V bias; fp8 DR V/FF (v3/v4).
"""

import numpy as np
import ml_dtypes

import concourse.bass as bass
import concourse.mybir as mybir
import concourse.tile as tile
from concourse import bacc
from concourse import bass_utils

# Problem shapes (hardcoded per contest contract).
B, S, D, H, DHID = 2, 2048, 1024, 16, 4096
NCORES = 8
GROUP = 4              # cores per batch group
HPC = 4                # heads per core
DK = 64                # q/k head dim
DV = 256               # v head dim
DKS = HPC * DK         # 256  k-slice per core
DVS = HPC * DV         # 1024 v/hidden slice per core
ROWS = S // GROUP      # 512  output rows per core after ReduceScatter
NM = D // 128          # 8    contraction chunks over d_model
VSTRIDE = DV + 1       # 257  V columns per head incl. ones column
NQC = 4                # 512-wide q chunks
WSCALE = 64.0          # host pre-scale on Wf/Wv for fp8
RG = [[0, 1, 2, 3], [4, 5, 6, 7]]

BF16 = mybir.dt.bfloat16
F32 = mybir.dt.float32
F8 = mybir.dt.float8e4
AF = mybir.ActivationFunctionType
DR = mybir.MatmulPerfMode.DoubleRow
MUL = mybir.AluOpType.mult
ADD = mybir.AluOpType.add

bf16 = ml_dtypes.bfloat16
f8e4 = ml_dtypes.float8_e4m3

_compiled = None


def build_program():
    nc = bacc.Bacc(
        "TRN2",
        target_bir_lowering=False,
        debug=False,
        enable_asserts=True,
        num_devices=NCORES,
    )

    # Per-core inputs (values differ per core; program is SPMD-identical).
    # Weight tensors arrive pre-arranged in their exact SBUF image
    # ([128, chunk-major]) so each bulk DMA is contiguous 2KB+ partition
    # lines -- the [D, x] row-major layouts were descriptor-bound (512B
    # lines, ~10us for 0.5MB).
    xT = nc.dram_tensor("xT", [D, S], BF16, kind="ExternalInput").ap()
    qT = nc.dram_tensor("qT", [128, 2 * S], BF16, kind="ExternalInput").ap()
    # residual rows (x+bf) for this core's rank blocks: 4 blocks of 128
    # rows, block g holds x rows 512g+128r+[0,128) -- added POST-RS.
    xres = nc.dram_tensor("xres", [ROWS, D], BF16, kind="ExternalInput").ap()
    wk = nc.dram_tensor("wk", [128, NM * DKS], BF16, kind="ExternalInput").ap()
    wv = nc.dram_tensor("wv8", [128, NM * DVS], F8, kind="ExternalInput").ap()
    wf8 = nc.dram_tensor("wf8", [128, NM * D], F8, kind="ExternalInput").ap()
    bkT = nc.dram_tensor("bkT", [128, 2], F32, kind="ExternalInput").ap()
    bvbc = nc.dram_tensor("bvbc", [128, DVS], BF16, kind="ExternalInput").ap()
    maskt = nc.dram_tensor("maskt", [128, 128], BF16, kind="ExternalInput").ap()
    ident = nc.dram_tensor("ident", [128, 128], BF16, kind="ExternalInput").ap()
    out = nc.dram_tensor("out", [ROWS, D], F32, kind="ExternalOutput").ap()

    with tile.TileContext(nc) as tc:
        _body(nc, tc, xT, qT, xres, wk, wv, wf8, bkT, bvbc, maskt, ident, out)

    nc.compile()
    return nc


def _body(nc, tc, xT, qT, xres, wk, wv, wf8, bkT, bvbc, maskt, ident, out):
    NST = S // 128     # 16 k tiles of 128
    from contextlib import ExitStack

    with ExitStack() as ctx:
        constp = ctx.enter_context(tc.tile_pool(name="const", bufs=1))
        kvp = ctx.enter_context(tc.tile_pool(name="kv", bufs=1))
        xtp = ctx.enter_context(tc.tile_pool(name="xt", bufs=1))
        wtsp = ctx.enter_context(tc.tile_pool(name="wts", bufs=1))
        expp = ctx.enter_context(tc.tile_pool(name="exp", bufs=1))
        gotbp = ctx.enter_context(tc.tile_pool(name="gotb", bufs=2))
        gotfp = ctx.enter_context(tc.tile_pool(name="gotf", bufs=2))
        otp = ctx.enter_context(tc.tile_pool(name="ot", bufs=4))
        fop = ctx.enter_context(tc.tile_pool(name="fo", bufs=2))
        resp = ctx.enter_context(tc.tile_pool(name="res", bufs=1))
        smallp = ctx.enter_context(tc.tile_pool(name="small", bufs=8))
        dramp = ctx.enter_context(tc.tile_pool(name="dram", bufs=1, space="DRAM"))
        # ---- tiny consts on Scalar (kept free for ACT work) -----------
        mask_sb = constp.tile([128, 128], BF16)
        nc.scalar.dma_start(mask_sb[:], maskt[:])
        ident_sb = constp.tile([128, 128], BF16)
        nc.scalar.dma_start(ident_sb[:], ident[:])
        bkT_sb = constp.tile([128, 2], F32)
        nc.scalar.dma_start(bkT_sb[:], bkT[:])

        # Warm up the collectives path (ncfw/channel setup) so the first
        # real ReduceScatter doesn't pay ~25us of first-call overhead.
        warm_in = dramp.tile([4, 16], BF16, tag="warm_in")
        warm_out = dramp.tile([1, 16], BF16, tag="warm_out")
        nc.scalar.dma_start(
            warm_in[:].rearrange("a b -> (a b)")[None, :], maskt[0:1, 0:64]
        )
        nc.gpsimd.collective_compute(
            "ReduceScatter",
            mybir.AluOpType.add,
            replica_groups=RG,
            ins=[warm_in.opt()],
            outs=[warm_out.opt()],
        )

        # ---- bulk loads on Sync, first-needed first -------------------
        wk_sb = wtsp.tile([128, NM * DKS], BF16, tag="wk")
        nc.sync.dma_start(wk_sb[:], wk[:])
        # xT chunk m lands right before the K-proj matmuls that need it;
        # the fp8 copy for the V projection is cast on-chip per chunk.
        xT_sb = xtp.tile([128, NM * S], BF16, tag="xT")
        x8T_sb = xtp.tile([128, NM * S], F8, tag="x8T")
        for m in range(NM):
            nc.sync.dma_start(
                xT_sb[:, m * S : (m + 1) * S], xT[m * 128 : (m + 1) * 128, :]
            )
            nc.vector.tensor_copy(
                x8T_sb[:, m * S : (m + 1) * S], xT_sb[:, m * S : (m + 1) * S]
            )
        qT_sb = kvp.tile([128, 2 * S], BF16)
        nc.sync.dma_start(qT_sb[:], qT[:])
        wv_sb = wtsp.tile([128, NM * DVS], F8, tag="wv")
        nc.sync.dma_start(wv_sb[:], wv[:])
        bv_sb = constp.tile([128, DVS], BF16)
        nc.sync.dma_start(bv_sb[:], bvbc[:])
        xres_sb = resp.tile([128, 4 * D], BF16)
        nc.sync.dma_start(
            xres_sb[:].rearrange("p (g d) -> p g d", g=4),
            xres.rearrange("(g p) d -> p g d", p=128),
        )
        wf8_sb = wtsp.tile([128, NM * D], F8, tag="wf8")
        nc.sync.dma_start(wf8_sb[:], wf8[:])
        kt_sb = kvp.tile([128, 2 * S], BF16)   # K^T rows dk%128, chunk dk//128
        v_sb = kvp.tile([128, NST * HPC * VSTRIDE], BF16)
        # softmax ones-columns (written once; V-proj drains skip them)
        nc.vector.memset(
            v_sb[:].rearrange("p (t h c) -> p t h c", t=NST, h=HPC)[:, :, :, DV],
            1.0,
        )

        # ---- K^T projection: chunk-major, overlaps the xT stream ------
        # bias bk added in the DVE drain (per-partition scalar), no bias MMs
        with tc.tile_pool(name="psK", bufs=4, space="PSUM") as psK:
            for dkt in range(2):
                pss = [psK.tile([128, 512], F32, tag="k", name="kps")
                       for _ in range(4)]
                for m in range(NM):
                    for st in range(4):
                        nc.tensor.matmul(
                            pss[st][:],
                            wk_sb[:, m * DKS + dkt * 128 : m * DKS + dkt * 128 + 128],
                            xT_sb[:, m * S + st * 512 : m * S + st * 512 + 512],
                            start=(m == 0),
                            stop=(m == NM - 1),
                        )
                for st in range(4):
                    nc.vector.tensor_scalar_add(
                        kt_sb[:, dkt * S + st * 512 : dkt * S + st * 512 + 512],
                        pss[st][:],
                        bkT_sb[:, dkt : dkt + 1],
                    )

        # ---- main pipeline --------------------------------------------
        if True:
            psV = ctx.enter_context(tc.tile_pool(name="psV", bufs=1, space="PSUM"))
            psS = ctx.enter_context(tc.tile_pool(name="psS", bufs=3, space="PSUM"))
            psA = ctx.enter_context(tc.tile_pool(name="psA", bufs=2, space="PSUM"))
            psF = ctx.enter_context(tc.tile_pool(name="psF", bufs=1, space="PSUM"))
            psT = ctx.enter_context(tc.tile_pool(name="psT", bufs=1, space="PSUM"))

            def v_tile(st, dvh):
                """V[s-tile, 512 dv cols] for heads (2dvh, 2dvh+1).
                fp8 DoubleRow; bias bv added in the DVE drain."""
                ps = psV.tile([128, 512], F32, tag="v")
                x8r = x8T_sb[:].rearrange("p (m s) -> p m s", m=NM)
                wvr = wv_sb[:].rearrange("p (m d) -> p m d", m=NM)
                for mp in range(NM // 2):
                    nc.tensor.matmul(
                        ps[:],
                        x8r[:, 2 * mp : 2 * mp + 2, st * 128 : st * 128 + 128],
                        wvr[:, 2 * mp : 2 * mp + 2, dvh * 512 : dvh * 512 + 512],
                        start=(mp == 0),
                        stop=(mp == NM // 2 - 1),
                        perf_mode=DR,
                    )
                base = st * HPC * VSTRIDE
                for hh in range(2):
                    h = 2 * dvh + hh
                    nc.vector.scalar_tensor_tensor(
                        v_sb[:, base + h * VSTRIDE : base + h * VSTRIDE + DV],
                        ps[:, hh * 256 : hh * 256 + 256],
                        1.0 / WSCALE,
                        bv_sb[:, dvh * 512 + hh * 256 : dvh * 512 + hh * 256 + 256],
                        op0=MUL,
                        op1=ADD,
                    )

            def sc_tile(pair, qc, kt, exps_t):
                """scores^T[k-tile kt, q chunk qc] for heads 2p, 2p+1.
                Both heads run concurrently in PE row quadrants into separate
                one-bank PSUM tiles, each drained by its own Exp."""
                t = kt - 4 * qc
                toff = max(t, 0) * 128
                er = exps_t[:].rearrange("p (t h w) -> p t h w", t=NST, h=2)
                for hl in range(2):
                    po = 64 * hl
                    ps = psS.tile([128, 512], F32, tag="s")
                    nc.tensor.matmul(
                        ps[:, toff:512],
                        kt_sb[po : po + 64,
                              pair * S + kt * 128 : pair * S + kt * 128 + 128],
                        qT_sb[po : po + 64,
                              pair * S + qc * 512 + toff : pair * S + qc * 512 + 512],
                        start=True,
                        stop=True,
                        tile_position=(po, 0),
                    )
                    nc.scalar.activation(
                        er[:, kt, hl, toff:512], ps[:, toff:512], AF.Exp
                    )
                if t >= 0:  # mask the diagonal 128x128 block of both heads
                    for hl in range(2):
                        blk = exps_t[:, kt * 1024 + hl * 512 + toff
                                     : kt * 1024 + hl * 512 + toff + 128]
                        nc.vector.tensor_mul(blk, blk, mask_sb[:])

            def av_tile(head, qc, sq, exps_t):
                """o[q-tile sq, dv] for head; returns normalized bf16 tile."""
                hl = head % 2
                pso = psA.tile([128, VSTRIDE], F32, tag="a")
                for kt in range(sq + 1):
                    vb = kt * HPC * VSTRIDE + head * VSTRIDE
                    eo = kt * 1024 + hl * 512 + (sq - 4 * qc) * 128
                    nc.tensor.matmul(
                        pso[:],
                        exps_t[:, eo : eo + 128],
                        v_sb[:, vb : vb + VSTRIDE],
                        start=(kt == 0),
                        stop=(kt == sq),
                    )
                recip = smallp.tile([128, 1], F32, tag="recip")
                nc.vector.reciprocal(recip[:], pso[:, DV : DV + 1])
                ot = otp.tile([128, DV], BF16, tag="ot")
                nc.vector.tensor_scalar_mul(ot[:], pso[:, 0:DV], recip[:])
                return ot

            def tp_tile(pair, qc, sq, ot0, ot1, gotb):
                """Transpose both heads' o tiles into gotb columns via PE
                transpose-mode (4x 128x128 into one bf16 PSUM tile, one DVE
                copy out)."""
                pt = psT.tile([128, 512], BF16, tag="t")
                for hh, ot in ((0, ot0), (1, ot1)):
                    for half in range(2):
                        nc.tensor.transpose(
                            pt[:, (2 * hh + half) * 128 : (2 * hh + half) * 128 + 128],
                            ot[:, half * 128 : half * 128 + 128],
                            ident_sb[:],
                        )
                qo = (sq - 4 * qc) * 128
                gbr = gotb[:].rearrange("p (h q) -> p h q", h=2 * HPC)
                nc.vector.tensor_copy(
                    gbr[:, 4 * pair : 4 * pair + 4, qo : qo + 128],
                    pt[:].rearrange("p (h q) -> p h q", h=4),
                )

            def ff_unit_for(g, gotf):
                """FF for q rows [512g, +512) in fp8 DoubleRow as 8 PSUM-tile
                sub-closures (qt, dmodel-half), plus the RS trigger."""
                gfr = gotf[:].rearrange("p (h q) -> p h q", h=2 * HPC)
                wfr = wf8_sb[:].rearrange("p (h d) -> p h d", h=NM)
                partial_d = dramp.tile([512, D], BF16, tag=f"part{g}",
                                       name="partial")
                fo_state = {}

                def sub(qt, half):
                    if half == 0:
                        fo_state[qt] = fop.tile([128, D], BF16, tag="fo",
                                                name="fo")
                    fo = fo_state[qt]
                    ps = psF.tile([128, 512], F32, tag="f", name="fps")
                    for dr in range(4):
                        nc.tensor.matmul(
                            ps[:],
                            gfr[:, 2 * dr : 2 * dr + 2, qt * 128 : qt * 128 + 128],
                            wfr[:, 2 * dr : 2 * dr + 2,
                                half * 512 : half * 512 + 512],
                            start=(dr == 0),
                            stop=(dr == 3),
                            perf_mode=DR,
                        )
                    nc.vector.tensor_scalar_mul(
                        fo[:, half * 512 : half * 512 + 512], ps[:], 1.0 / WSCALE
                    )
                    if half == 1:
                        nc.sync.dma_start(
                            partial_d[qt * 128 : (qt + 1) * 128, :], fo[:]
                        )

                def trigger():
                    rs_d = dramp.tile([128, D], BF16, tag=f"rs{g}", name="rsd")
                    nc.gpsimd.collective_compute(
                        "ReduceScatter",
                        mybir.AluOpType.add,
                        replica_groups=RG,
                        ins=[partial_d.opt()],
                        outs=[rs_d.opt()],
                    )
                    return rs_d

                return sub, trigger

            def drain_unit(g, rs_d):
                """RS(g) output -> +residual -> final rows [128g, +128)."""
                rs_sb = smallp.tile([128, D], BF16, tag="rs_sb", bufs=2,
                                    name="rssb")
                nc.gpsimd.dma_start(rs_sb[:], rs_d[:])
                res_sb = smallp.tile([128, D], F32, tag="res_sb", bufs=2,
                                     name="ressb")
                nc.vector.tensor_add(
                    res_sb[:], rs_sb[:], xres_sb[:, g * D : (g + 1) * D]
                )
                nc.gpsimd.dma_start(out[g * 128 : (g + 1) * 128, :], res_sb[:])

            # ---- the pipeline ----
            # Two exp buffers alternate per pair phase; buffer B reuses the
            # xT staging slot (its readers -- K-proj matmuls and the fp8
            # casts -- precede all attention in engine program order).
            exps_A = expp.tile([128, NST * 1024], BF16, tag="e")
            exps_B = xtp.tile([128, NST * 1024], BF16, tag="xT", name="exps_B")
            ebufs = [exps_A, exps_B]
            pairs = [(qc, p) for qc in range(NQC) for p in range(2)]
            lead_done = [0] * len(pairs)   # leading strips emitted per pair

            def emit_lead(i, n):
                """Emit the next n leading-score strips of pair i."""
                qc, p = pairs[i]
                nlead = 4 * qc + 1
                while n > 0 and lead_done[i] < nlead:
                    sc_tile(p, qc, lead_done[i], ebufs[i % 2])
                    lead_done[i] += 1
                    n -= 1

            gotbs, gotfs = [], []
            pend_gelu = []
            ff_subs, ff_trigs, rs_ds = {}, {}, {}

            def gelu_half(gf, gb, h):
                gfv = gf[:].rearrange("p (h q) -> p h q", h=2 * HPC)
                gbv = gb[:].rearrange("p (h q) -> p h q", h=2 * HPC)
                nc.scalar.activation(
                    gfv[:, :, 256 * h : 256 * h + 256],
                    gbv[:, :, 256 * h : 256 * h + 256],
                    AF.Gelu,
                )

            def flush_gelu():
                for gf, gb in pend_gelu:
                    gelu_half(gf, gb, 0)
                    gelu_half(gf, gb, 1)
                pend_gelu.clear()

            for i, (qc, p) in enumerate(pairs):
                eb = ebufs[i % 2]
                last = i == len(pairs) - 1
                if p == 0:
                    gotb = gotbp.tile([128, 2 * HPC * 512], BF16, tag="gotb")
                    gotf = gotfp.tile([128, 2 * HPC * 512], F8, tag="gotf")
                    gotbs.append(gotb)
                    gotfs.append(gotf)
                    if qc >= 1:
                        s, t = ff_unit_for(qc - 1, gotfs[qc - 1])
                        ff_subs[qc - 1], ff_trigs[qc - 1] = s, t
                # --- filler phase: any remaining own leading strips, then
                # gelus + V tiles + previous group's FF sub-units (PE-solid,
                # ACT-light: next-pair exps were drained during the previous
                # AV phase) ---
                emit_lead(i, 99)
                if p == 0:
                    flush_gelu()
                for st in range(4 * qc, 4 * qc + 4):
                    v_tile(st, p)
                if qc >= 1:
                    sub = ff_subs[qc - 1]
                    for qt in (2 * p, 2 * p + 1):
                        for half in range(2):
                            sub(qt, half)
                    if p == 1:
                        # partial(qc-1) complete: trigger its RS now (before
                        # any later trigger) and drain the RS from 2 groups
                        # back, keeping the gpsimd queue pipelined
                        rs_ds[qc - 1] = ff_trigs[qc - 1]()
                        if qc >= 2:
                            drain_unit(qc - 2, rs_ds[qc - 2])
                # --- AV phase, interleaving pair i+1's leading scores ---
                if last:
                    sF, tF = ff_unit_for(NQC - 1, gotf)
                nxt_total = 0 if last else 4 * pairs[i + 1][0] + 1
                for j, sq in enumerate(range(4 * qc, 4 * qc + 4)):
                    if sq < 4 * qc + 3:
                        sc_tile(p, qc, sq + 1, eb)
                    ot0 = av_tile(2 * p + 0, qc, sq, eb)
                    ot1 = av_tile(2 * p + 1, qc, sq, eb)
                    tp_tile(p, qc, sq, ot0, ot1, gotb)
                    if not last:
                        tgt = (j + 1) * nxt_total // 4
                        emit_lead(i + 1, tgt - lead_done[i + 1])
                    elif j >= 1:
                        # last pair: q-half 0 of the final chunk is complete
                        # after sq=13 (both pairs' sq 12,13 done) -> gelu,
                        # then its FF sub-units one AV slot later so the
                        # gelu latency hides behind AV
                        if j == 1:
                            gelu_half(gotf, gotb, 0)
                        else:
                            for half in range(2):
                                sF(j - 2, half)
                if last:
                    gelu_half(gotf, gotb, 1)
                    for half in range(2):
                        sF(1, half)
                    for qt in (2, 3):
                        for half in range(2):
                            sF(qt, half)
                    rs_ds[NQC - 1] = tF()
                if p == 1 and not last:
                    pend_gelu.append((gotf, gotb))
            drain_unit(NQC - 2, rs_ds[NQC - 2])
            drain_unit(NQC - 1, rs_ds[NQC - 1])


def make_in_maps(x, Wk, bk, Wv, bv, Wf, bf):
    """Host-side sharding: returns the per-core input dict list."""
    x = np.asarray(x, np.float32)
    Wk = np.asarray(Wk, np.float32)
    Wv = np.asarray(Wv, np.float32)
    Wf = np.asarray(Wf, np.float32)
    bk = np.asarray(bk, np.float32)
    bv = np.asarray(bv, np.float32)
    bf = np.asarray(bf, np.float32)
    mask = np.tril(np.ones((128, 128), np.float32)).T  # mask[k,q]=1 iff k<=q
    in_maps = []
    for c in range(NCORES):
        b, r = c // GROUP, c % GROUP
        xb = x[b]                                    # [S, D]
        xT = np.ascontiguousarray(xb.T).astype(bf16)
        qTs = xT[DKS * r : DKS * (r + 1)]            # heads 4r..4r+3 rows
        # compact residual rows: block g holds x rows 512g+128r+[0,128)
        xres = np.empty((ROWS, D), np.float32)
        for g in range(4):
            rows = slice(512 * g + 128 * r, 512 * g + 128 * r + 128)
            xres[128 * g : 128 * g + 128] = xb[rows] + bf[None, :]
        bkTv = np.ascontiguousarray(
            bk[DKS * r : DKS * (r + 1)].reshape(2, 128).T
        ).astype(np.float32)
        wvs = Wv[:, DVS * r : DVS * (r + 1)]

        def sbuf_image(a):
            """[NM*128, X] chunk-major -> SBUF image [128, NM*X]."""
            n, xw = a.shape[0] // 128, a.shape[1]
            return np.ascontiguousarray(
                a.reshape(n, 128, xw).transpose(1, 0, 2).reshape(128, n * xw)
            )

        m = {
            "xT": xT,
            "qT": sbuf_image(np.asarray(qTs)),
            "xres": xres.astype(bf16),
            "wk": sbuf_image(Wk[:, DKS * r : DKS * (r + 1)].astype(bf16)),
            "wv8": sbuf_image((wvs * WSCALE).astype(f8e4)),
            "wf8": sbuf_image((Wf[DVS * r : DVS * (r + 1), :] * WSCALE).astype(f8e4)),
            "bkT": bkTv,
            "bvbc": np.broadcast_to(
                bv[None, DVS * r : DVS * (r + 1)], (128, DVS)
            ).astype(bf16),
            "maskt": mask.astype(bf16),
            "ident": np.eye(128, dtype=np.float32).astype(bf16),
        }
        in_maps.append(m)
    return in_maps


def assemble(results):
    """[8 x [512,1024]] core outputs -> [2,2048,1024]."""
    out = np.empty((B, S, D), np.float32)
    for c in range(NCORES):
        b, r = c // GROUP, c % GROUP
        for g in range(4):
            out[b, 512 * g + 128 * r : 512 * g + 128 * r + 128, :] = results[c][
                "out"
            ][128 * g : 128 * (g + 1)]
    return out


def kernel(x, Wk, bk, Wv, bv, Wf, bf, _trace=False, _trace_cores=None):
    global _compiled
    if _compiled is None:
        _compiled = build_program()
    nc = _compiled
    in_maps = make_in_maps(x, Wk, bk, Wv, bv, Wf, bf)
    res = bass_utils.run_bass_kernel_spmd(
        nc,
        in_maps,
        core_ids=list(range(NCORES)),
        trace=_trace,
        trace_cores=_trace_cores,
    )
    out = assemble(res.results)
    kernel.last_result = res
    return out


# revision 21
# speedup vs baseline: 1.0180x; 1.0180x over previous
"""Trainium2 Bass kernel for nn_ExperimentalLayer9 (dense transformer layer).

Layer: x + gelu(attn(x)) @ Wf with
  Q = split_heads(x), K = split_heads(x@Wk+bk), V = split_heads(x@Wv+bv)
  causal softmax (no 1/sqrt(d) scale), exact-erf gelu, residual add.

Sharding over 8 NeuronCores: 2 batch groups x 4-way head/tensor parallel.
Core c handles batch b=c//4 and heads [4r, 4r+4) with r=c%4.

v5 pipeline (vs the v4 349us run):
  * Fully software-pipelined pair phases: the leading score strips of pair
    i+1 are emitted inside pair i's AV slots, with the exp buffer double-
    buffered (the second buffer reuses the xT staging slot, dead after the
    K projection + fp8 casts).  The per-pair "leading phase" is then pure
    PE filler (V tiles + previous group's FF sub-units + gelus on ACT),
    so the PE no longer stalls on ACT exp pacing -- v4's HAM traces showed
    131us at K=4/8 from exactly those bubbles.
  * All bulk loads moved to the Sync queue (wk first): multi-us DMA
    descriptor-generation slices were blocking the Scalar queue where the
    exps run; scalar keeps only 3 tiny const loads.
  * The last chunk's FF sub-units + RS trigger are folded into its AV
    phase half-by-half, shortening the tail.
  * 4 full-group 1MB ReduceScatters, drains software-pipelined one group
    behind (v4); residual added post-collective from a compact [512,D]
    slice; PE transpose-mode o^T; DVE-fused K/V bias; fp8 DR V/FF (v3/v4).
"""

import numpy as np
import ml_dtypes

import concourse.bass as bass
import concourse.mybir as mybir
import concourse.tile as tile
from concourse import bacc
from concourse import bass_utils

# Problem shapes (hardcoded per contest contract).
B, S, D, H, DHID = 2, 2048, 1024, 16, 4096
NCORES = 8
GROUP = 4              # cores per batch group
HPC = 4                # heads per core
DK = 64                # q/k head dim
DV = 256               # v head dim
DKS = HPC * DK         # 256  k-slice per core
DVS = HPC * DV         # 1024 v/hidden slice per core
ROWS = S // GROUP      # 512  output rows per core after ReduceScatter
NM = D // 128          # 8    contraction chunks over d_model
VSTRIDE = DV + 1       # 257  V columns per head incl. ones column
NQC = 4                # 512-wide q chunks
WSCALE = 64.0          # host pre-scale on Wf/Wv for fp8
RG = [[0, 1, 2, 3], [4, 5, 6, 7]]

BF16 = mybir.dt.bfloat16
F32 = mybir.dt.float32
F8 = mybir.dt.float8e4
AF = mybir.ActivationFunctionType
DR = mybir.MatmulPerfMode.DoubleRow
MUL = mybir.AluOpType.mult
ADD = mybir.AluOpType.add

bf16 = ml_dtypes.bfloat16
f8e4 = ml_dtypes.float8_e4m3

_compiled = None


def build_program():
    nc = bacc.Bacc(
        "TRN2",
        target_bir_lowering=False,
        debug=False,
        enable_asserts=True,
        num_devices=NCORES,
    )

    # Per-core inputs (values differ per core; program is SPMD-identical).
    # Weight tensors arrive pre-arranged in their exact SBUF image
    # ([128, chunk-major]) so each bulk DMA is contiguous 2KB+ partition
    # lines -- the [D, x] row-major layouts were descriptor-bound (512B
    # lines, ~10us for 0.5MB).
    xT = nc.dram_tensor("xT", [D, S], BF16, kind="ExternalInput").ap()
    qT = nc.dram_tensor("qT", [128, 2 * S], BF16, kind="ExternalInput").ap()
    # residual rows (x+bf) for this core's rank blocks: 4 blocks of 128
    # rows, block g holds x rows 512g+128r+[0,128) -- added POST-RS.
    xres = nc.dram_tensor("xres", [ROWS, D], BF16, kind="ExternalInput").ap()
    wk = nc.dram_tensor("wk", [128, NM * DKS], BF16, kind="ExternalInput").ap()
    wv = nc.dram_tensor("wv8", [128, NM * DVS], F8, kind="ExternalInput").ap()
    wf8 = nc.dram_tensor("wf8", [128, NM * D], F8, kind="ExternalInput").ap()
    bkT = nc.dram_tensor("bkT", [128, 2], F32, kind="ExternalInput").ap()
    bvbc = nc.dram_tensor("bvbc", [128, DVS], BF16, kind="ExternalInput").ap()
    maskt = nc.dram_tensor("maskt", [128, 128], BF16, kind="ExternalInput").ap()
    ident = nc.dram_tensor("ident", [128, 128], BF16, kind="ExternalInput").ap()
    out = nc.dram_tensor("out", [ROWS, D], F32, kind="ExternalOutput").ap()

    with tile.TileContext(nc) as tc:
        _body(nc, tc, xT, qT, xres, wk, wv, wf8, bkT, bvbc, maskt, ident, out)

    nc.compile()
    return nc


def _body(nc, tc, xT, qT, xres, wk, wv, wf8, bkT, bvbc, maskt, ident, out):
    NST = S // 128     # 16 k tiles of 128
    from contextlib import ExitStack

    with ExitStack() as ctx:
        constp = ctx.enter_context(tc.tile_pool(name="const", bufs=1))
        kvp = ctx.enter_context(tc.tile_pool(name="kv", bufs=1))
        xtp = ctx.enter_context(tc.tile_pool(name="xt", bufs=1))
        wtsp = ctx.enter_context(tc.tile_pool(name="wts", bufs=1))
        expp = ctx.enter_context(tc.tile_pool(name="exp", bufs=1))
        gotbp = ctx.enter_context(tc.tile_pool(name="gotb", bufs=2))
        gotfp = ctx.enter_context(tc.tile_pool(name="gotf", bufs=2))
        otp = ctx.enter_context(tc.tile_pool(name="ot", bufs=4))
        fop = ctx.enter_context(tc.tile_pool(name="fo", bufs=2))
        resp = ctx.enter_context(tc.tile_pool(name="res", bufs=1))
        smallp = ctx.enter_context(tc.tile_pool(name="small", bufs=8))
        dramp = ctx.enter_context(tc.tile_pool(name="dram", bufs=1, space="DRAM"))
        # ---- tiny consts on Scalar (kept free for ACT work) -----------
        mask_sb = constp.tile([128, 128], BF16)
        nc.scalar.dma_start(mask_sb[:], maskt[:])
        ident_sb = constp.tile([128, 128], BF16)
        nc.scalar.dma_start(ident_sb[:], ident[:])
        bkT_sb = constp.tile([128, 2], F32)
        nc.scalar.dma_start(bkT_sb[:], bkT[:])

        # Warm up the collectives path (ncfw/channel setup) so the first
        # real ReduceScatter doesn't pay ~25us of first-call overhead.
        warm_in = dramp.tile([4, 16], BF16, tag="warm_in")
        warm_out = dramp.tile([1, 16], BF16, tag="warm_out")
        nc.scalar.dma_start(
            warm_in[:].rearrange("a b -> (a b)")[None, :], maskt[0:1, 0:64]
        )
        nc.gpsimd.collective_compute(
            "ReduceScatter",
            mybir.AluOpType.add,
            replica_groups=RG,
            ins=[warm_in.opt()],
            outs=[warm_out.opt()],
        )

        # ---- bulk loads on Sync, first-needed first -------------------
        wk_sb = wtsp.tile([128, NM * DKS], BF16, tag="wk")
        nc.sync.dma_start(wk_sb[:], wk[:])
        # xT chunk m lands right before the K-proj matmuls that need it;
        # the fp8 copy for the V projection is cast on-chip per chunk.
        xT_sb = xtp.tile([128, NM * S], BF16, tag="xT")
        x8T_sb = xtp.tile([128, NM * S], F8, tag="x8T")
        for m in range(NM):
            nc.sync.dma_start(
                xT_sb[:, m * S : (m + 1) * S], xT[m * 128 : (m + 1) * 128, :]
            )
            nc.vector.tensor_copy(
                x8T_sb[:, m * S : (m + 1) * S], xT_sb[:, m * S : (m + 1) * S]
            )
        qT_sb = kvp.tile([128, 2 * S], BF16)
        nc.sync.dma_start(qT_sb[:], qT[:])
        wv_sb = wtsp.tile([128, NM * DVS], F8, tag="wv")
        nc.sync.dma_start(wv_sb[:], wv[:])
        bv_sb = constp.tile([128, DVS], BF16)
        nc.sync.dma_start(bv_sb[:], bvbc[:])
        xres_sb = resp.tile([128, 4 * D], BF16)
        nc.sync.dma_start(
            xres_sb[:].rearrange("p (g d) -> p g d", g=4),
            xres.rearrange("(g p) d -> p g d", p=128),
        )
        wf8_sb = wtsp.tile([128, NM * D], F8, tag="wf8")
        nc.sync.dma_start(wf8_sb[:], wf8[:])
        kt_sb = kvp.tile([128, 2 * S], BF16)   # K^T rows dk%128, chunk dk//128
        v_sb = kvp.tile([128, NST * HPC * VSTRIDE], BF16)
        # softmax ones-columns (written once; V-proj drains skip them)
        nc.vector.memset(
            v_sb[:].rearrange("p (t h c) -> p t h c", t=NST, h=HPC)[:, :, :, DV],
            1.0,
        )

        # ---- K^T projection: chunk-major, overlaps the xT stream ------
        # bias bk added in the DVE drain (per-partition scalar), no bias MMs
        with tc.tile_pool(name="psK", bufs=4, space="PSUM") as psK:
            for dkt in range(2):
                pss = [psK.tile([128, 512], F32, tag="k", name="kps")
                       for _ in range(4)]
                for m in range(NM):
                    for st in range(4):
                        nc.tensor.matmul(
                            pss[st][:],
                            wk_sb[:, m * DKS + dkt * 128 : m * DKS + dkt * 128 + 128],
                            xT_sb[:, m * S + st * 512 : m * S + st * 512 + 512],
                            start=(m == 0),
                            stop=(m == NM - 1),
                        )
                for st in range(4):
                    nc.vector.tensor_scalar_add(
                        kt_sb[:, dkt * S + st * 512 : dkt * S + st * 512 + 512],
                        pss[st][:],
                        bkT_sb[:, dkt : dkt + 1],
                    )

        # ---- main pipeline --------------------------------------------
        if True:
            psV = ctx.enter_context(tc.tile_pool(name="psV", bufs=1, space="PSUM"))
            psS = ctx.enter_context(tc.tile_pool(name="psS", bufs=3, space="PSUM"))
            psA = ctx.enter_context(tc.tile_pool(name="psA", bufs=2, space="PSUM"))
            psF = ctx.enter_context(tc.tile_pool(name="psF", bufs=1, space="PSUM"))
            psT = ctx.enter_context(tc.tile_pool(name="psT", bufs=1, space="PSUM"))

            def v_tile(st, dvh):
                """V[s-tile, 512 dv cols] for heads (2dvh, 2dvh+1).
                fp8 DoubleRow; bias bv added in the DVE drain."""
                ps = psV.tile([128, 512], F32, tag="v")
                x8r = x8T_sb[:].rearrange("p (m s) -> p m s", m=NM)
                wvr = wv_sb[:].rearrange("p (m d) -> p m d", m=NM)
                for mp in range(NM // 2):
                    nc.tensor.matmul(
                        ps[:],
                        x8r[:, 2 * mp : 2 * mp + 2, st * 128 : st * 128 + 128],
                        wvr[:, 2 * mp : 2 * mp + 2, dvh * 512 : dvh * 512 + 512],
                        start=(mp == 0),
                        stop=(mp == NM // 2 - 1),
                        perf_mode=DR,
                    )
                base = st * HPC * VSTRIDE
                for hh in range(2):
                    h = 2 * dvh + hh
                    nc.vector.scalar_tensor_tensor(
                        v_sb[:, base + h * VSTRIDE : base + h * VSTRIDE + DV],
                        ps[:, hh * 256 : hh * 256 + 256],
                        1.0 / WSCALE,
                        bv_sb[:, dvh * 512 + hh * 256 : dvh * 512 + hh * 256 + 256],
                        op0=MUL,
                        op1=ADD,
                    )

            def sc_tile(pair, qc, kt, exps_t):
                """scores^T[k-tile kt, q chunk qc] for heads 2p, 2p+1.
                Both heads run concurrently in PE row quadrants into separate
                one-bank PSUM tiles, each drained by its own Exp."""
                t = kt - 4 * qc
                toff = max(t, 0) * 128
                er = exps_t[:].rearrange("p (t h w) -> p t h w", t=NST, h=2)
                for hl in range(2):
                    po = 64 * hl
                    ps = psS.tile([128, 512], F32, tag="s")
                    nc.tensor.matmul(
                        ps[:, toff:512],
                        kt_sb[po : po + 64,
                              pair * S + kt * 128 : pair * S + kt * 128 + 128],
                        qT_sb[po : po + 64,
                              pair * S + qc * 512 + toff : pair * S + qc * 512 + 512],
                        start=True,
                        stop=True,
                        tile_position=(po, 0),
                    )
                    nc.scalar.activation(
                        er[:, kt, hl, toff:512], ps[:, toff:512], AF.Exp
                    )
                if t >= 0:  # mask the diagonal 128x128 block of both heads
                    for hl in range(2):
                        blk = exps_t[:, kt * 1024 + hl * 512 + toff
                                     : kt * 1024 + hl * 512 + toff + 128]
                        nc.vector.tensor_mul(blk, blk, mask_sb[:])

            def av_tile(head, qc, sq, exps_t):
                """o[q-tile sq, dv] for head; returns normalized bf16 tile."""
                hl = head % 2
                pso = psA.tile([128, VSTRIDE], F32, tag="a")
                for kt in range(sq + 1):
                    vb = kt * HPC * VSTRIDE + head * VSTRIDE
                    eo = kt * 1024 + hl * 512 + (sq - 4 * qc) * 128
                    nc.tensor.matmul(
                        pso[:],
                        exps_t[:, eo : eo + 128],
                        v_sb[:, vb : vb + VSTRIDE],
                        start=(kt == 0),
                        stop=(kt == sq),
                    )
                recip = smallp.tile([128, 1], F32, tag="recip")
                nc.vector.reciprocal(recip[:], pso[:, DV : DV + 1])
                ot = otp.tile([128, DV], BF16, tag="ot")
                nc.vector.tensor_scalar_mul(ot[:], pso[:, 0:DV], recip[:])
                return ot

            def tp_tile(pair, qc, sq, ot0, ot1, gotb):
                """Transpose both heads' o tiles into gotb columns via PE
                transpose-mode (4x 128x128 into one bf16 PSUM tile, one DVE
                copy out)."""
                pt = psT.tile([128, 512], BF16, tag="t")
                for hh, ot in ((0, ot0), (1, ot1)):
                    for half in range(2):
                        nc.tensor.transpose(
                            pt[:, (2 * hh + half) * 128 : (2 * hh + half) * 128 + 128],
                            ot[:, half * 128 : half * 128 + 128],
                            ident_sb[:],
                        )
                qo = (sq - 4 * qc) * 128
                gbr = gotb[:].rearrange("p (h q) -> p h q", h=2 * HPC)
                nc.vector.tensor_copy(
                    gbr[:, 4 * pair : 4 * pair + 4, qo : qo + 128],
                    pt[:].rearrange("p (h q) -> p h q", h=4),
                )

            def ff_unit_for(g, gotf):
                """FF for q rows [512g, +512) in fp8 DoubleRow as 8 PSUM-tile
                sub-closures (qt, dmodel-half), plus the RS trigger."""
                gfr = gotf[:].rearrange("p (h q) -> p h q", h=2 * HPC)
                wfr = wf8_sb[:].rearrange("p (h d) -> p h d", h=NM)
                partial_d = dramp.tile([512, D], BF16, tag=f"part{g}",
                                       name="partial")
                fo_state = {}

                def sub(qt, half):
                    if half == 0:
                        fo_state[qt] = fop.tile([128, D], BF16, tag="fo",
                                                name="fo")
                    fo = fo_state[qt]
                    ps = psF.tile([128, 512], F32, tag="f", name="fps")
                    for dr in range(4):
                        nc.tensor.matmul(
                            ps[:],
                            gfr[:, 2 * dr : 2 * dr + 2, qt * 128 : qt * 128 + 128],
                            wfr[:, 2 * dr : 2 * dr + 2,
                                half * 512 : half * 512 + 512],
                            start=(dr == 0),
                            stop=(dr == 3),
                            perf_mode=DR,
                        )
                    nc.vector.tensor_scalar_mul(
                        fo[:, half * 512 : half * 512 + 512], ps[:], 1.0 / WSCALE
                    )
                    if half == 1:
                        nc.sync.dma_start(
                            partial_d[qt * 128 : (qt + 1) * 128, :], fo[:]
                        )

                def trigger():
                    rs_d = dramp.tile([128, D], BF16, tag=f"rs{g}", name="rsd")
                    nc.gpsimd.collective_compute(
                        "ReduceScatter",
                        mybir.AluOpType.add,
                        replica_groups=RG,
                        ins=[partial_d.opt()],
                        outs=[rs_d.opt()],
                    )
                    return rs_d

                return sub, trigger

            def drain_unit(g, rs_d):
                """RS(g) output -> +residual -> final rows [128g, +128).
                Entirely on GpSimd (the collective-waiting queue): a DVE add
                here would head-of-line-block the attention drains behind
                the RS completion."""
                rs_sb = smallp.tile([128, D], BF16, tag="rs_sb", bufs=2,
                                    name="rssb")
                nc.gpsimd.dma_start(rs_sb[:], rs_d[:])
                res_sb = smallp.tile([128, D], F32, tag="res_sb", bufs=2,
                                     name="ressb")
                nc.gpsimd.tensor_add(
                    res_sb[:], rs_sb[:], xres_sb[:, g * D : (g + 1) * D]
                )
                nc.gpsimd.dma_start(out[g * 128 : (g + 1) * 128, :], res_sb[:])

            # ---- the pipeline ----
            # Two exp buffers alternate per pair phase; buffer B reuses the
            # xT staging slot (its readers -- K-proj matmuls and the fp8
            # casts -- precede all attention in engine program order).
            exps_A = expp.tile([128, NST * 1024], BF16, tag="e")
            exps_B = xtp.tile([128, NST * 1024], BF16, tag="xT", name="exps_B")
            ebufs = [exps_A, exps_B]
            pairs = [(qc, p) for qc in range(NQC) for p in range(2)]
            lead_done = [0] * len(pairs)   # leading strips emitted per pair

            def emit_lead(i, n):
                """Emit the next n leading-score strips of pair i."""
                qc, p = pairs[i]
                nlead = 4 * qc + 1
                while n > 0 and lead_done[i] < nlead:
                    sc_tile(p, qc, lead_done[i], ebufs[i % 2])
                    lead_done[i] += 1
                    n -= 1

            gotbs, gotfs = [], []
            pend_gelu = []
            ff_subs, ff_trigs, rs_ds = {}, {}, {}

            def gelu_part(gf, gb, h, n=2):
                """Gelu over 1/n of the q columns (h-th part)."""
                w = 512 // n
                gfv = gf[:].rearrange("p (h q) -> p h q", h=2 * HPC)
                gbv = gb[:].rearrange("p (h q) -> p h q", h=2 * HPC)
                nc.scalar.activation(
                    gfv[:, :, w * h : w * h + w],
                    gbv[:, :, w * h : w * h + w],
                    AF.Gelu,
                )

            def flush_gelu():
                for gf, gb in pend_gelu:
                    gelu_part(gf, gb, 0)
                    gelu_part(gf, gb, 1)
                pend_gelu.clear()

            for i, (qc, p) in enumerate(pairs):
                eb = ebufs[i % 2]
                last = i == len(pairs) - 1
                if p == 0:
                    gotb = gotbp.tile([128, 2 * HPC * 512], BF16, tag="gotb")
                    gotf = gotfp.tile([128, 2 * HPC * 512], F8, tag="gotf")
                    gotbs.append(gotb)
                    gotfs.append(gotf)
                    if qc >= 1:
                        s, t = ff_unit_for(qc - 1, gotfs[qc - 1])
                        ff_subs[qc - 1], ff_trigs[qc - 1] = s, t
                # --- filler phase: any remaining own leading strips, then
                # gelus + V tiles + previous group's FF sub-units (PE-solid,
                # ACT-light: next-pair exps were drained during the previous
                # AV phase) ---
                emit_lead(i, 99)
                if p == 0:
                    flush_gelu()
                for st in range(4 * qc, 4 * qc + 4):
                    v_tile(st, p)
                if qc >= 1:
                    sub = ff_subs[qc - 1]
                    for qt in (2 * p, 2 * p + 1):
                        for half in range(2):
                            sub(qt, half)
                    if p == 1:
                        # partial(qc-1) complete: trigger its RS now (before
                        # any later trigger) and drain the RS from 2 groups
                        # back, keeping the gpsimd queue pipelined
                        rs_ds[qc - 1] = ff_trigs[qc - 1]()
                        if qc >= 2:
                            drain_unit(qc - 2, rs_ds[qc - 2])
                # --- AV phase, interleaving pair i+1's leading scores ---
                if last:
                    sF, tF = ff_unit_for(NQC - 1, gotf)
                nxt_total = 0 if last else 4 * pairs[i + 1][0] + 1
                for j, sq in enumerate(range(4 * qc, 4 * qc + 4)):
                    if sq < 4 * qc + 3:
                        sc_tile(p, qc, sq + 1, eb)
                    ot0 = av_tile(2 * p + 0, qc, sq, eb)
                    ot1 = av_tile(2 * p + 1, qc, sq, eb)
                    tp_tile(p, qc, sq, ot0, ot1, gotb)
                    if not last:
                        tgt = (j + 1) * nxt_total // 4
                        emit_lead(i + 1, tgt - lead_done[i + 1])
                    elif j >= 1:
                        # last pair: q-quarter qt of the final chunk is
                        # complete after sq=12+qt (both pairs) -> quarter
                        # gelus as soon as ready, FF sub-units one AV slot
                        # later so each gelu's latency hides behind AV
                        gelu_part(gotf, gotb, j - 1, n=4)
                        if j >= 2:
                            for half in range(2):
                                sF(j - 2, half)
                if last:
                    gelu_part(gotf, gotb, 3, n=4)
                    for qt in (2, 3):
                        for half in range(2):
                            sF(qt, half)
                    rs_ds[NQC - 1] = tF()
                if p == 1 and not last:
                    pend_gelu.append((gotf, gotb))
            drain_unit(NQC - 2, rs_ds[NQC - 2])
            drain_unit(NQC - 1, rs_ds[NQC - 1])


def make_in_maps(x, Wk, bk, Wv, bv, Wf, bf):
    """Host-side sharding: returns the per-core input dict list."""
    x = np.asarray(x, np.float32)
    Wk = np.asarray(Wk, np.float32)
    Wv = np.asarray(Wv, np.float32)
    Wf = np.asarray(Wf, np.float32)
    bk = np.asarray(bk, np.float32)
    bv = np.asarray(bv, np.float32)
    bf = np.asarray(bf, np.float32)
    mask = np.tril(np.ones((128, 128), np.float32)).T  # mask[k,q]=1 iff k<=q
    in_maps = []
    for c in range(NCORES):
        b, r = c // GROUP, c % GROUP
        xb = x[b]                                    # [S, D]
        xT = np.ascontiguousarray(xb.T).astype(bf16)
        qTs = xT[DKS * r : DKS * (r + 1)]            # heads 4r..4r+3 rows
        # compact residual rows: block g holds x rows 512g+128r+[0,128)
        xres = np.empty((ROWS, D), np.float32)
        for g in range(4):
            rows = slice(512 * g + 128 * r, 512 * g + 128 * r + 128)
            xres[128 * g : 128 * g + 128] = xb[rows] + bf[None, :]
        bkTv = np.ascontiguousarray(
            bk[DKS * r : DKS * (r + 1)].reshape(2, 128).T
        ).astype(np.float32)
        wvs = Wv[:, DVS * r : DVS * (r + 1)]

        def sbuf_image(a):
            """[NM*128, X] chunk-major -> SBUF image [128, NM*X]."""
            n, xw = a.shape[0] // 128, a.shape[1]
            return np.ascontiguousarray(
                a.reshape(n, 128, xw).transpose(1, 0, 2).reshape(128, n * xw)
            )

        m = {
            "xT": xT,
            "qT": sbuf_image(np.asarray(qTs)),
            "xres": xres.astype(bf16),
            "wk": sbuf_image(Wk[:, DKS * r : DKS * (r + 1)].astype(bf16)),
            "wv8": sbuf_image((wvs * WSCALE).astype(f8e4)),
            "wf8": sbuf_image((Wf[DVS * r : DVS * (r + 1), :] * WSCALE).astype(f8e4)),
            "bkT": bkTv,
            "bvbc": np.broadcast_to(
                bv[None, DVS * r : DVS * (r + 1)], (128, DVS)
            ).astype(bf16),
            "maskt": mask.astype(bf16),
            "ident": np.eye(128, dtype=np.float32).astype(bf16),
        }
        in_maps.append(m)
    return in_maps


def assemble(results):
    """[8 x [512,1024]] core outputs -> [2,2048,1024]."""
    out = np.empty((B, S, D), np.float32)
    for c in range(NCORES):
        b, r = c // GROUP, c % GROUP
        for g in range(4):
            out[b, 512 * g + 128 * r : 512 * g + 128 * r + 128, :] = results[c][
                "out"
            ][128 * g : 128 * (g + 1)]
    return out


def kernel(x, Wk, bk, Wv, bv, Wf, bf, _trace=False, _trace_cores=None):
    global _compiled
    if _compiled is None:
        _compiled = build_program()
    nc = _compiled
    in_maps = make_in_maps(x, Wk, bk, Wv, bv, Wf, bf)
    res = bass_utils.run_bass_kernel_spmd(
        nc,
        in_maps,
        core_ids=list(range(NCORES)),
        trace=_trace,
        trace_cores=_trace_cores,
    )
    out = assemble(res.results)
    kernel.last_result = res
    return out
